# revision 1
# baseline (speedup 1.0000x reference)
"""Trainium2 Bass kernel for nn_MultiHeadAttention_7954279432294.

Reference computation (per batch b, row h):
    qp = q^T Wq^T + bq       [W, C]   (1x1 conv channel mixing)
    kp = k^T Wk^T + bk       [W, C]
    vp = v^T Wv^T + bv       [W, C]
    out = (qp @ kp^T) @ vp   [W, C]   (linear attention, NO softmax)
    result = out^T + q       [C, W]   (NCHW + residual)

Optimizations over v1 (which did everything in fp32, 4 PE-cycles/row,
~206us measured, PE-bound):

1. Reassociation (kept from v1): out = qp @ (kp^T @ vp), S = kp^T @ vp is
   only [C, C].
2. q-side folding: out^T = M^T q + t 1^T + residual, where
   M^T = Wq_cm^T-contracted with S (M^T[i,c'] = sum_c Wq[c,i] S[c,c']) and
   t = S^T bq. The q projection never materializes: the raw fp32 q feeds
   the final [64x64] @ [64x512] matmul directly as float32r (1 cycle/row
   at N=512, full fp32 operand precision).
3. fp16 for the k/v side: k,v are converted once per chunk on the ACT
   engine; pv projection + S run at 1 cycle/row. PSUM accumulates fp32.
4. pv projection restructured to M=128 stationary tiles: 4 matmuls of
   N=128 per h (vs v1's 8 of N=128 per h) by computing kp|vp packed in
   [w, 128ch] layout with a block-diagonal [[Wk^T,0],[0,Wv^T]] rhs.

5. The two per-h O^T matmuls are merged into ONE K=128 float32r matmul
   with a block-diagonal M^T (f32r matmuls require PSUM dst partition 0;
   the off-diagonal zero blocks live in two persistent SBUF tiles).
6. Queue separation: input DMAs on the SP queue, const DMAs on the ACT
   queue, output DMAs on the Pool queue (issued per half-chunk), so no
   DMA issue ever waits behind another queue's semaphores.
7. The residual (+q) is folded into the O^T matmul by adding I to the
   block-diagonal M^T during its eviction (tensor_tensor with a host-
   sent identity tile, cost-neutral on DVE): the entire residual pass
   (one op + one engine hop per pair, 35us/rep of GPSIMD time that
   paced the drain tail) disappears.

Per-pair PE rows: pv 8x128 + S 8x64 + M 2x64 + t 2 + out 1x512 = 2178
at 1 cyc/row -> ~30us PE busy vs ~93us DMA (32 MiB @ 360 GB/s): the
kernel is DMA-bound, as the problem's memory target_regime intends.

Measured (differential over in-NEFF reps, interleaved blocks, 8 cores):
  v1 baseline (fp32, PE-bound):   205.6 us/rep
  this kernel:                    89.2 us/rep median of 6 sessions
                                  (82.8-91.3 range; rel err 4.8e-4)
  pure-DMA floor, same pattern:   ~92-96 us/rep
TimelineSim: 89.9 us/rep steady-state, 109.8 us one-shot, DMA engines
94% busy.

Sharding: data-parallel over B (8 batches over 8 cores), weights
replicated, no cross-device communication.

Layout trick kept from v1: H=64 rows processed as 32 pairs (h, h+8)
packed into the 128 SBUF partitions for the K=64 matmuls.
"""

import numpy as np

import concourse.bass as bass
import concourse.mybir as mybir
import concourse.tile as tile
from concourse.bass_utils import run_bass_kernel_spmd

B, C, H, W = 8, 64, 64, 512
HW = H * W
F32 = mybir.dt.float32
F32R = mybir.dt.float32r
F16 = mybir.dt.float16

# chunking: 4 chunks of 16 h-rows; each chunk tile is [128, 8*512] = 2 MB
N_CHUNK = 4
H_PER_CHUNK = H // N_CHUNK          # 16
PAIRS_PER_CHUNK = H_PER_CHUNK // 2  # 8
CHUNK_F = PAIRS_PER_CHUNK * W       # 4096


def _add_bcast(nc, out_ap, in0_ap, bias_tile, reps, width):
    """out = in0 + bias, where bias is a [128, width] tile broadcast `reps`
    times along the free dim (out/in0 are [128, reps*width])."""
    out3 = out_ap.rearrange("p (r c) -> p r c", c=width)
    in03 = in0_ap.rearrange("p (r c) -> p r c", c=width)
    b2 = bias_tile[:, :]
    bias3 = bass.AP(
        tensor=b2.tensor,
        offset=b2.offset,
        ap=[b2.ap[0], [0, reps], b2.ap[1]],
    )
    nc.vector.tensor_tensor(
        out=out3, in0=in03, in1=bias3, op=mybir.AluOpType.add
    )


def build_nc(hw_workaround: bool = False, reps: int = 1) -> bass.Bass:
    """reps>1 repeats the whole computation inside the NEFF (idempotent) —
    used only for differential HW timing (launch overhead cancels)."""
    nc = bass.Bass()

    # weights are preprocessed host-side in kernel():
    #   Wkv -> fp16 block-diag [[Wk^T, 0], [0, Wv^T]] -> [128, 128]
    #   bkv -> fp32, every partition = concat(bk, bv) -> [128, 128]
    #   Wq  -> fp16 Wq as [o(=c), i] duplicated on both halves -> [128, C]
    #   bq  -> fp16 per-partition column duplicated -> [128, 1]
    q_d = nc.declare_dram_parameter("q", [C, HW], F32R, isOutput=False)
    k_d = nc.declare_dram_parameter("k", [C, HW], F32, isOutput=False)
    v_d = nc.declare_dram_parameter("v", [C, HW], F32, isOutput=False)
    Wkv_d = nc.declare_dram_parameter("Wkv", [128, 128], F16, isOutput=False)
    bkv_d = nc.declare_dram_parameter("bkv", [128, 128], F32, isOutput=False)
    Wq_d = nc.declare_dram_parameter("Wq", [128, C], F16, isOutput=False)
    id_d = nc.declare_dram_parameter("ident", [128, C], F32, isOutput=False)
    bq_d = nc.declare_dram_parameter("bq", [128, 1], F16, isOutput=False)
    out_d = nc.declare_dram_parameter("out", [C, HW], F32, isOutput=True)

    # chunk ch, g-half: DRAM region x[c, ch*8192 + g*4096 + e] maps to SBUF
    # partitions g*64+c. One [64, 4096] DMA per (tensor, chunk, half).
    def dram_half(d, ch, g):
        lo = ch * 2 * CHUNK_F + g * CHUNK_F
        return d[:, lo : lo + CHUNK_F]

    with tile.TileContext(nc) as tc:
        with (
            tc.tile_pool(name="const", bufs=1) as const,
            tc.tile_pool(name="io", bufs=2) as io,
            tc.tile_pool(name="cvt", bufs=2) as cvt,
            tc.tile_pool(name="mid", bufs=2) as mid,
            tc.tile_pool(name="psA", bufs=2, space="PSUM") as psA,
            tc.tile_pool(name="psS", bufs=2, space="PSUM") as psS,
            tc.tile_pool(name="psO", bufs=2, space="PSUM") as psO,
        ):
            # ---------------- setup: plain DMAs (host did the prep) ----------
            # const DMAs ride the ACT queue so the SP queue starts on
            # chunk-0 input DMAs immediately (consts land well before the
            # first convert finishes).
            wkv = const.tile([128, 128], F16)
            nc.scalar.dma_start(out=wkv[:, :], in_=Wkv_d[:, :])

            bkv = const.tile([128, 128], F32)
            nc.scalar.dma_start(out=bkv[:, :], in_=bkv_d[:, :])

            wq = const.tile([128, C], F16)
            nc.scalar.dma_start(out=wq[:, :], in_=Wq_d[:, :])

            bq = const.tile([128, 1], F16)
            nc.scalar.dma_start(out=bq[:, :], in_=bq_d[:, :])

            ident = const.tile([128, C], F32)
            nc.scalar.dma_start(out=ident[:, :], in_=id_d[:, :])

            # block-diag M^T [128,128] f32r for the single K=128 O^T matmul
            # (f32r matmuls require PSUM dst partition base 0). Off-diagonal
            # blocks are zeroed once; per-pair evicts rewrite the diagonal
            # blocks. Two tiles, manually alternated (hp%2) for pipelining.
            mtbd0 = const.tile([128, 128], F32R)
            mtbd1 = const.tile([128, 128], F32R)
            for mtbd in (mtbd0, mtbd1):
                nc.vector.tensor_scalar_mul(mtbd[0:C, C:128], bkv[0:C, 0:C], 0.0)
                nc.vector.tensor_scalar_mul(mtbd[C:128, 0:C], bkv[0:C, 0:C], 0.0)

            # ---------------- main loop ----------------
            # tile_position safety (empirical, from v1): consecutive matmuls
            # may not switch tile rows unless row == col. All matmuls here
            # are (0, 0), (0, 64) or (64, 64).
            lo, hi = slice(0, C), slice(C, 128)
            for ch in [c for _ in range(reps) for c in range(N_CHUNK)]:
                # kv{g}_sb rows 0:64 = k channels, 64:128 = v channels (h-grp g)
                kv0_sb = io.tile([128, CHUNK_F], F32, tag="kv0_sb")
                kv1_sb = io.tile([128, CHUNK_F], F32, tag="kv1_sb")
                q_sb = io.tile([128, CHUNK_F], F32R, tag="q_sb", bufs=3)
                o_sb = io.tile([128, CHUNK_F], F32, tag="o_sb", bufs=3)
                for g, kv_sb in ((0, kv0_sb), (1, kv1_sb)):
                    nc.sync.dma_start(out=kv_sb[lo, :], in_=dram_half(k_d, ch, g))
                    nc.sync.dma_start(out=kv_sb[hi, :], in_=dram_half(v_d, ch, g))
                for g in range(2):
                    gp = slice(g * C, (g + 1) * C)
                    nc.sync.dma_start(out=q_sb[gp, :], in_=dram_half(q_d, ch, g))

                # fp16 copies of k|v for the PE (ACT engine, one op per tile)
                kvB0 = cvt.tile([128, CHUNK_F], F16, tag="kvB0")
                kvB1 = cvt.tile([128, CHUNK_F], F16, tag="kvB1")
                hf = CHUNK_F // 2
                nc.scalar.copy(kvB0[:, 0:hf], kv0_sb[:, 0:hf])
                nc.scalar.copy(kvB1[:, 0:hf], kv1_sb[:, 0:hf])
                nc.scalar.copy(kvB0[:, hf:], kv0_sb[:, hf:])
                nc.scalar.copy(kvB1[:, hf:], kv1_sb[:, hf:])

                for hp in range(PAIRS_PER_CHUNK):
                    hs = slice(hp * W, (hp + 1) * W)

                    # --- pv: kp|vp packed [w, 128ch], one [128,512] per g ---
                    # lhsT (stationary) = kv fp16 slice [128(c), 128(w)],
                    # rhs (moving) = block-diag Wkv [128(c), 128(ch)].
                    pv_ps0 = psA.tile([128, W], F32, tag="pv_ps0")
                    pv_ps1 = psA.tile([128, W], F32, tag="pv_ps1")
                    for kvB, pv_ps in ((kvB0, pv_ps0), (kvB1, pv_ps1)):
                        for j in range(4):
                            ws = slice(hp * W + j * 128, hp * W + (j + 1) * 128)
                            nc.tensor.matmul(
                                pv_ps[:, j * 128 : (j + 1) * 128],
                                kvB[:, ws], wkv[:, :],
                                start=True, stop=True,
                            )
                    # evict + bias -> fp16 (DVE)
                    pvB0 = mid.tile([128, W], F16, tag="pvB0")
                    pvB1 = mid.tile([128, W], F16, tag="pvB1")
                    _add_bcast(nc, pvB0[:, :], pv_ps0[:, :], bkv, 4, width=128)
                    _add_bcast(nc, pvB1[:, :], pv_ps1[:, :], bkv, 4, width=128)

                    # --- S = kp^T @ vp [c, c'] per h (4 accum K=128 blocks);
                    # then M^T[i,c'] = sum_c Wq[c,i] S[c,c'], t = S^T bq.
                    # S, M^T, t share one PSUM bank: cols 0:64, 64:128, 128. ---
                    smt_ps = psS.tile([128, 132], F32, tag="smt_ps")
                    for g, pvB in ((0, pvB0), (1, pvB1)):
                        gp = slice(g * C, (g + 1) * C)
                        for j in range(4):
                            nc.tensor.matmul(
                                smt_ps[gp, 0:C],
                                pvB[:, j * 128 : j * 128 + C],
                                pvB[:, j * 128 + C : (j + 1) * 128],
                                start=(j == 0), stop=(j == 3),
                            )
                    S_sb = mid.tile([128, C], F16, tag="S_sb")
                    nc.vector.tensor_scalar_add(S_sb[:, :], smt_ps[:, 0:C], 0.0)

                    for g in range(2):
                        gp = slice(g * C, (g + 1) * C)
                        nc.tensor.matmul(
                            smt_ps[gp, C : 2 * C], wq[gp, :], S_sb[gp, :],
                            start=True, stop=True,
                        )
                        nc.tensor.matmul(
                            smt_ps[gp, 2 * C : 2 * C + 1],
                            S_sb[gp, :], bq[gp, :],
                            start=True, stop=True,
                        )
                    # The +I folds the residual (+q) into the O^T matmul:
                    # out^T[c',w] = sum_k (MT+I)[k,c'] q[k,w] = attn^T + q.
                    # The whole residual pass (one op + one engine hop per
                    # pair) disappears; the identity add rides the eviction.
                    mtbd = (mtbd0, mtbd1)[hp % 2]
                    t_sb = mid.tile([128, 1], F32, tag="t_sb")
                    nc.vector.tensor_tensor(
                        out=mtbd[0:C, 0:C], in0=smt_ps[0:C, C : 2 * C],
                        in1=ident[0:C, :], op=mybir.AluOpType.add)
                    nc.vector.tensor_tensor(
                        out=mtbd[C:128, C:128], in0=smt_ps[C:128, C : 2 * C],
                        in1=ident[C:128, :], op=mybir.AluOpType.add)
                    nc.vector.tensor_scalar_add(t_sb[:, :], smt_ps[:, 2 * C : 2 * C + 1], 0.0)

                    # --- out^T[c',w] = sum_i M^T[i,c'] q[i,w]: ONE K=128
                    # float32r matmul (block-diag M^T), raw f32r q moving,
                    # N=512 -> 1 cyc/row, PSUM dst partition base 0. ---
                    out_ps = psO.tile([128, W], F32, tag="out_ps")
                    nc.tensor.matmul(
                        out_ps[:, :], mtbd[:, :], q_sb[:, hs],
                        start=True, stop=True,
                    )

                    # --- assemble: +t only (ACT, PSUM->SBUF); residual is
                    # already inside out_ps via the +I fold above ---
                    nc.scalar.add(o_sb[:, hs], out_ps[:, :], add=t_sb[:, :])

                    # out-DMAs ride the Pool queue (never block SP's input
                    # DMAs), split per half-chunk so they drain early.
                    if hp in (3, PAIRS_PER_CHUNK - 1):
                        half = hp // 4
                        hw_lo = half * (CHUNK_F // 2)
                        hsl = slice(hw_lo, hw_lo + CHUNK_F // 2)
                        for g in range(2):
                            gp = slice(g * C, (g + 1) * C)
                            d = dram_half(out_d, ch, g)
                            nc.gpsimd.dma_start(
                                out=d[:, hsl], in_=o_sb[gp, hsl]
                            )

    if hw_workaround:
        _absorb_matmul_waits(nc)
    nc.finalize()
    return nc


def _absorb_matmul_waits(nc):
    """This walrus build rejects any engine instruction carrying more than one
    sync wait. Split an instruction's n waits into n same-engine NoOps (one
    wait each) inserted right before it: engines execute their stream in FIFO
    order, so the instruction stays correctly gated."""
    ctr = 0
    for bb in nc.m.functions[0].blocks:
        insts = bb.instructions
        i = 0
        while i < len(insts):
            inst = insts[i]
            si = inst.sync_info
            if si is not None and si.on_wait and len(si.on_wait) > 1:
                for w in si.on_wait:
                    nop = mybir.InstNoOp(
                        name=f"I-mmwait-{ctr}", engine=inst.engine, ins=[], outs=[]
                    )
                    ctr += 1
                    nop.sync_info = mybir.SyncInfo(on_wait=[w], on_update=[])
                    insts.insert(i, nop)
                    i += 1
                inst.sync_info = mybir.SyncInfo(
                    on_wait=[], on_update=list(si.on_update)
                )
            i += 1


_NC_CACHE = None
_RUN_KWARGS = {}   # test harness can set e.g. {"trace": True}
LAST_RESULT = None  # BassKernelResults of the last kernel() call


def _get_nc():
    global _NC_CACHE
    if _NC_CACHE is None:
        # the 1-wait workaround is needed for the HW compile path only;
        # CoreSim/TimelineSim consume a clean build_nc() module.
        _NC_CACHE = build_nc(hw_workaround=True)
    return _NC_CACHE


def prep_params(Wq, bq, Wk, bk, Wv, bv):
    Wq = np.asarray(Wq, dtype=np.float32)
    Wk = np.asarray(Wk, dtype=np.float32)
    Wv = np.asarray(Wv, dtype=np.float32)
    bq = np.asarray(bq, dtype=np.float32).reshape(C)
    bk = np.asarray(bk, dtype=np.float32).reshape(C)
    bv = np.asarray(bv, dtype=np.float32).reshape(C)

    # fp16 block-diag [[Wk^T, 0], [0, Wv^T]] -> [128, 128]
    Wkv = np.zeros((128, 128), dtype=np.float16)
    Wkv[0:C, 0:C] = Wk.T.astype(np.float16)
    Wkv[C:128, C:128] = Wv.T.astype(np.float16)
    # fp32, every partition = concat(bk, bv) -> [128, 128]
    bkv = np.ascontiguousarray(
        np.tile(np.concatenate([bk, bv]).reshape(1, 128), (128, 1))
    )
    # fp16 Wq as [o(=c), i] duplicated on both halves -> [128, C]
    Wq_p = np.ascontiguousarray(np.concatenate([Wq, Wq], axis=0)).astype(np.float16)
    # fp16 bq column duplicated -> [128, 1]
    bq_p = np.ascontiguousarray(np.tile(bq.reshape(C, 1), (2, 1))).astype(np.float16)
    # identity blocks for the residual fold -> [128, C] f32
    ident = np.ascontiguousarray(np.tile(np.eye(C, dtype=np.float32), (2, 1)))
    return {"Wkv": Wkv, "bkv": bkv, "Wq": Wq_p, "bq": bq_p, "ident": ident}


def kernel(q, k, v, Wq, bq, Wk, bk, Wv, bv):
    q = np.ascontiguousarray(np.asarray(q), dtype=np.float32)
    k = np.ascontiguousarray(np.asarray(k), dtype=np.float32)
    v = np.ascontiguousarray(np.asarray(v), dtype=np.float32)
    params = prep_params(Wq, bq, Wk, bk, Wv, bv)

    nc = _get_nc()
    in_maps = []
    for b in range(B):
        in_maps.append(
            {
                "q": q[b].reshape(C, HW),
                "k": k[b].reshape(C, HW),
                "v": v[b].reshape(C, HW),
                **params,
            }
        )
    res = run_bass_kernel_spmd(nc, in_maps, list(range(B)), **_RUN_KWARGS)
    global LAST_RESULT
    LAST_RESULT = res
    out = np.stack([res.results[b]["out"].reshape(C, H, W) for b in range(B)])
    return out

